# revision 1
# baseline (speedup 1.0000x reference)
"""Trainium2 Bass kernel for batched multi-head attention.

Problem: N=8, S=1024, E=1024, H=16, DK=64 MultiHeadAttention with a boolean
attention mask, fp32 reference.

Strategy: pure batch data-parallelism -- one batch element per NeuronCore
(8 cores), weights replicated, no collectives.  Per core everything is
computed in a transposed layout so no on-chip transposes are needed:

  xT [E, S] (host-transposed)  --Wq/Wk-->  QT, KT [E, S]
  xT                           --Wv---->   V    [S, E]  (head-major, with a
                                                         ones column per head)
  logitsT[k, q] = KT_h^T-slices @ QT_h    (PSUM, fp32 accum)
  Em = exp(logitsT/8) * (1 - maskT)       (ACT exp -> fp16, DVE mask multiply)
  O_h[d|sum, q] = V_aug_h^T @ Em_h        (fp16 matmul; row 64 = softmax sums)
  oT[e', q] = O_h[0:64] * (1/sums)        (DVE; 1/sums broadcast across
                                           partitions via a DRAM bounce)
  out[q, e] = oT^T-slices @ Wo + bo_eff   (bo_eff = bv@Wo + bo folded on host)
"""

import numpy as np
from contextlib import ExitStack

import concourse.bass as bass
import concourse.mybir as mybir
import concourse.tile as tile
from concourse.vector_clock import ScopedClock
from concourse.bass_utils import run_bass_kernel_spmd

F32 = mybir.dt.float32
F32R = mybir.dt.float32r
BF16 = mybir.dt.bfloat16
F16 = mybir.dt.float16
U8 = mybir.dt.uint8
Exp = mybir.ActivationFunctionType.Exp
Ident = mybir.ActivationFunctionType.Identity
Copy = mybir.ActivationFunctionType.Copy
MULT = mybir.AluOpType.mult

N, S, E, H, DK = 8, 1024, 1024, 16, 64
P = 128
NT = E // P
NPAIR = H // 2

# dtype of the big matmul operands (projections and output projection).
# F16: full-rate matmuls with hidden weight loads.  F32R: ~TF32 precision,
# but self-loading weights make each matmul ~25% slower.
MM_DT = F16


# ---------------------------------------------------------------------------
# Workaround: this walrus build supports at most ONE semaphore wait per
# instruction.  Split instructions carrying more waits into NOP(wait) chains
# on the same engine, and do the same for the TileContext tail drain.
# ---------------------------------------------------------------------------
_MAXW = 1
# instruction types whose lowered ISA struct was observed to accept 2 waits
_MAXW2_TYPES = ()
_orig_lower = tile.TileContext._lower_ordered_insts
_tilefix_installed = False


def _split_waits(ordered):
    for _bb, insts in ordered.items():
        out = []
        for inst in insts:
            si = inst.sync_info
            maxw = 2 if type(inst).__name__ in _MAXW2_TYPES else _MAXW
            if si is not None and len(si.on_wait) > maxw:
                waits = list(si.on_wait)
                keep, extra = waits[:maxw], waits[maxw:]
                for i in range(0, len(extra), _MAXW):
                    out.append(
                        mybir.InstNoOp(
                            name=f"{inst.name}-ws{i}",
                            engine=inst.engine,
                            bass_nofuse=True,
                            sync_info=mybir.SyncInfo(
                                on_wait=extra[i : i + _MAXW], on_update=[]
                            ),
                        )
                    )
                inst.sync_info = mybir.SyncInfo(
                    on_wait=keep, on_update=list(si.on_update)
                )
            out.append(inst)
        insts[:] = out


def _patched_lower(self, ordered):
    _split_waits(ordered)
    return _orig_lower(self, ordered)


def _patched_drain_and_barrier(self, tick_clock, wait_clock):
    nc = self.nc
    drain_inst = nc.sync.drain()
    wait_clock.add_sem_waits(
        drain_inst.ins, ScopedClock({None: tick_clock.global_clock})
    )
    si = drain_inst.ins.sync_info
    waits = list(si.on_wait) if si is not None else []
    if len(waits) > _MAXW:
        drain_inst.ins.sync_info = mybir.SyncInfo(on_wait=[], on_update=[])
        for i in range(0, len(waits), _MAXW):
            nop = nc.sync.nop(nofuse=True)
            nop.ins.sync_info = mybir.SyncInfo(
                on_wait=waits[i : i + _MAXW], on_update=[]
            )
    nc.all_engine_barrier()
    popped = nc._tile_sem_poison_stack.pop()
    assert popped is self._sem_poison
    nc.clear_and_free_semaphores(list(self.sems.allocated().values()))
    nc.all_engine_barrier()


def _install_tilefix():
    global _tilefix_installed
    if not _tilefix_installed:
        tile.TileContext._lower_ordered_insts = _patched_lower
        tile.TileContext._drain_and_barrier = _patched_drain_and_barrier
        _tilefix_installed = True


# ---------------------------------------------------------------------------
# Kernel build
# ---------------------------------------------------------------------------
_cached_nc = None


def _build(repeat=1, mm_dt=None, mask_pair=True):
    global _cached_nc
    if _cached_nc is not None and repeat == 1 and mm_dt is None and mask_pair:
        return _cached_nc
    if mm_dt is None:
        mm_dt = MM_DT
    _install_tilefix()

    nc = bass.Bass("TRN2", num_devices=N)

    x_t = nc.declare_dram_parameter("x_t", [E, S], mm_dt, isOutput=False)
    mask_t = nc.declare_dram_parameter("mask_t", [S, S], U8, isOutput=False)
    wq = nc.declare_dram_parameter("wq", [E, E], mm_dt, isOutput=False)
    wk = nc.declare_dram_parameter("wk", [E, E], mm_dt, isOutput=False)
    wv = nc.declare_dram_parameter("wv", [E, E], mm_dt, isOutput=False)
    wo = nc.declare_dram_parameter("wo", [E, E], mm_dt, isOutput=False)
    bqc = nc.declare_dram_parameter("bqc", [P, NT], F32, isOutput=False)
    bkc = nc.declare_dram_parameter("bkc", [P, NT], F32, isOutput=False)
    bo_eff = nc.declare_dram_parameter("bo_eff", [E], F32, isOutput=False)
    out = nc.declare_dram_parameter("out", [S, E], F32, isOutput=True)

    def tiled(ap):
        return ap.rearrange("(t p) f -> p t f", p=P)

    x_tt = tiled(x_t.ap())
    mask_tt = tiled(mask_t.ap())
    w_t = {
        "q": tiled(wq.ap()),
        "k": tiled(wk.ap()),
        "v": tiled(wv.ap()),
        "o": tiled(wo.ap()),
    }
    out_t = tiled(out.ap())

    with tile.TileContext(nc) as tc, ExitStack() as ctx:
        # persistent pools, longest lifetime first (stack allocator)
        p_oT = ctx.enter_context(tc.tile_pool(name="oT", bufs=1))
        p_pers = ctx.enter_context(tc.tile_pool(name="pers", bufs=1))

        oT = p_oT.tile([P, NT, S], mm_dt)
        QT = p_pers.tile([P, NT, S], mm_dt)
        KT = p_pers.tile([P, NT, S], mm_dt)
        Vg = p_pers.tile([P, NT, H * (DK + 1)], F16)
        nm = p_pers.tile([P, NT, S], F16)
        bq_sb = p_pers.tile([P, NT], F32)
        bk_sb = p_pers.tile([P, NT], F32)
        bo_sb = p_pers.tile([P, S], F32)
        p_woful = ctx.enter_context(tc.tile_pool(name="wofull", bufs=1))
        Wof = p_woful.tile([P, NT, S], mm_dt)

        for rep in range(repeat):
            nc.sync.dma_start(bq_sb[:], bqc[:])
            nc.sync.dma_start(bk_sb[:], bkc[:])
            nc.sync.dma_start(
                bo_sb[:],
                bo_eff.ap().rearrange("(o e) -> o e", o=1).broadcast_to((P, S)),
            )
            nc.any.memset(Vg[:, :, DK :: DK + 1], 1.0)
            ones64 = p_pers.tile([1, DK], F32, name=f"ones64_{rep}")
            nc.any.memset(ones64[:], 1.0)

            # ---- phase A: xT load, mask convert, Q/K/V projections ----
            with tc.tile_pool(name="phAw", bufs=1) as p_w, \
                 tc.tile_pool(name="phA", bufs=1) as p_x, \
                 tc.tile_pool(name="phAm", bufs=2) as p_m, \
                 tc.tile_pool(name="psA", bufs=8, space="PSUM") as psA:

                Wf = {
                    pr: p_w.tile([P, NT, S], mm_dt, name=f"Wf_{rep}_{pr}")
                    for pr in ("v", "q", "k")
                }
                xT = p_x.tile([P, NT, S], mm_dt, name=f"xT_{rep}")
                # interleave so the first projection's operands arrive first
                for j in range(NT):
                    nc.sync.dma_start(xT[:, j, :], x_tt[:, j, :])
                    nc.sync.dma_start(Wf["v"][:, j, :], w_t["v"][:, j, :])
                for pr in ("q", "k"):
                    for j in range(NT):
                        nc.sync.dma_start(Wf[pr][:, j, :], w_t[pr][:, j, :])
                for j in range(NT):
                    nc.sync.dma_start(Wof[:, j, :], w_t["o"][:, j, :])

                for j in range(NT):
                    mu = p_m.tile([P, S], U8, tag="mu", name=f"mu_{rep}_{j}")
                    nc.sync.dma_start(mu[:], mask_tt[:, j, :])
                    # notm = 1 - mask  (fp16; exact for 0/1)
                    nc.scalar.activation(
                        nm[:, j, :], mu[:], Ident, bias=1.0, scale=-1.0
                    )

                for proj in ("v", "q", "k"):
                    for half in range(2):
                        accs = {}
                        for j in range(NT):
                            wt = Wf[proj][:, j, :]
                            for ti in range(4):
                                t = half * 4 + ti
                                for c in range(2):
                                    if j == 0:
                                        accs[(t, c)] = psA.tile(
                                            [P, 512], F32, tag="acc",
                                            name=f"acc_{rep}_{proj}_{half}_{t}_{c}",
                                        )
                                    if proj == "v":
                                        lhsT = xT[:, j, t * P : (t + 1) * P]
                                        rhs = wt[:, c * 512 : (c + 1) * 512]
                                    else:
                                        lhsT = wt[:, t * P : (t + 1) * P]
                                        rhs = xT[:, j, c * 512 : (c + 1) * 512]
                                    nc.tensor.matmul(
                                        accs[(t, c)][:], lhsT, rhs,
                                        start=(j == 0), stop=(j == NT - 1),
                                    )
                        for ti in range(4):
                            t = half * 4 + ti
                            for c in range(2):
                                acc = accs[(t, c)]
                                if proj == "q":
                                    nc.vector.tensor_scalar_add(
                                        QT[:, t, c * 512 : (c + 1) * 512],
                                        acc[:], bq_sb[:, t : t + 1],
                                    )
                                elif proj == "k":
                                    nc.vector.tensor_scalar_add(
                                        KT[:, t, c * 512 : (c + 1) * 512],
                                        acc[:], bk_sb[:, t : t + 1],
                                    )
                                else:
                                    dst = Vg[
                                        :, t,
                                        c * 8 * (DK + 1) : (c + 1) * 8 * (DK + 1),
                                    ].rearrange("p (h d) -> p h d", d=DK + 1)[
                                        :, :, 0:DK
                                    ]
                                    src = acc[:].rearrange(
                                        "p (h d) -> p h d", d=DK
                                    )
                                    nc.scalar.activation(dst, src, Copy)

            # ---- phase B: attention per head pair ----
            with tc.tile_pool(name="phBe", bufs=6) as p_em, \
                 tc.tile_pool(name="phBs", bufs=6) as p_os, \
                 tc.tile_pool(name="phBr", bufs=8) as p_r, \
                 tc.tile_pool(name="phBrb", bufs=3) as p_rb, \
                 tc.tile_pool(name="phBd", bufs=4, space="DRAM") as p_dram, \
                 tc.tile_pool(name="psL", bufs=2, space="PSUM") as psL, \
                 tc.tile_pool(name="psO", bufs=2, space="PSUM") as psO:

                for p in range(NPAIR):
                    O = [
                        psO.tile([DK + 1, S], F32, tag="O", name=f"O_{rep}_{p}_{i}")
                        for i in range(2)
                    ]
                    for j in range(NT):
                        L = [
                            psL.tile([P, S], F32, tag="L", name=f"L_{rep}_{p}_{j}_{i}")
                            for i in range(2)
                        ]
                        for c in range(2):
                            for h in range(2):
                                r0, r1 = h * DK, h * DK + DK
                                nc.tensor.matmul(
                                    L[h][:, c * 512 : c * 512 + 512],
                                    KT[r0:r1, p, j * P : (j + 1) * P],
                                    QT[r0:r1, p, c * 512 : (c + 1) * 512],
                                    start=True, stop=True,
                                )
                        if mask_pair:
                            Emp = p_em.tile(
                                [P, 2 * S], F16, tag="Em", name=f"Em_{rep}_{p}_{j}"
                            )
                            Ems = [Emp[:, 0:S], Emp[:, S : 2 * S]]
                            for h in range(2):
                                nc.scalar.activation(
                                    Ems[h], L[h][:], Exp, scale=0.125
                                )
                            nc.vector.tensor_tensor(
                                Emp[:].rearrange("p (h q) -> p h q", h=2),
                                Emp[:].rearrange("p (h q) -> p h q", h=2),
                                nm[:, j : j + 1, :].broadcast_to((P, 2, S)),
                                MULT,
                            )
                        else:
                            Ems = [
                                p_em.tile(
                                    [P, S], F16, tag="Em", name=f"Em_{rep}_{p}_{j}_{i}"
                                )
                                for i in range(2)
                            ]
                            for h in range(2):
                                nc.scalar.activation(
                                    Ems[h][:], L[h][:], Exp, scale=0.125
                                )
                                # plain 2D step-1 operands keep the DVE in 2x mode
                                nc.vector.tensor_tensor(
                                    Ems[h][:], Ems[h][:], nm[:, j, :], MULT
                                )
                        for h in range(2):
                            head = 2 * p + h
                            for c in range(2):
                                nc.tensor.matmul(
                                    O[h][:, c * 512 : (c + 1) * 512],
                                    Vg[:, j, head * (DK + 1) : (head + 1) * (DK + 1)],
                                    Ems[h][:, c * 512 : (c + 1) * 512],
                                    start=(j == 0), stop=(j == NT - 1),
                                )
                    Rb = p_rb.tile([P, S], F32, tag="Rb", name=f"Rb_{rep}_{p}")
                    Ost = p_os.tile([P, S], F32, tag="Ost", name=f"Ost_{rep}_{p}")
                    for h in range(2):
                        # stage O out of PSUM fast so the accumulator banks
                        # free quickly; DVE so ACT stays on the exp stream
                        nc.vector.tensor_copy(
                            Ost[h * DK : (h + 1) * DK, :], O[h][0:DK, :]
                        )
                        R = p_r.tile([1, S], F32, tag="R", name=f"R_{rep}_{p}_{h}")
                        nc.vector.reciprocal(R[:], O[h][DK : DK + 1, :])
                        Rd = p_dram.tile([1, S], F32, tag="Rd", name=f"Rd_{rep}_{p}_{h}")
                        nc.gpsimd.dma_start(Rd[:], R[:])
                        nc.gpsimd.dma_start(
                            Rb[h * DK : (h + 1) * DK, :],
                            Rd[:].broadcast_to((DK, S)),
                        )
                        nc.vector.tensor_tensor(
                            oT[h * DK : (h + 1) * DK, p, :],
                            Ost[h * DK : (h + 1) * DK, :],
                            Rb[h * DK : (h + 1) * DK, :],
                            MULT,
                        )

            # ---- phase C: output projection ----
            with tc.tile_pool(name="phCo", bufs=2) as p_out, \
                 tc.tile_pool(name="psC", bufs=4, space="PSUM") as psC:
                for half in range(2):
                    F = {}
                    for j in range(NT):
                        wt = Wof[:, j, :]
                        for ti in range(4):
                            t = half * 4 + ti
                            if j == 0:
                                F[t] = psC.tile(
                                    [P, S], F32, tag="F", name=f"F_{rep}_{half}_{t}"
                                )
                            for c in range(2):
                                nc.tensor.matmul(
                                    F[t][:, c * 512 : (c + 1) * 512],
                                    oT[:, j, t * P : (t + 1) * P],
                                    wt[:, c * 512 : (c + 1) * 512],
                                    start=(j == 0), stop=(j == NT - 1),
                                )
                    for ti in range(4):
                        t = half * 4 + ti
                        ot = p_out.tile(
                            [P, S], F32, tag="ot", name=f"ot_{rep}_{half}_{ti}"
                        )
                        nc.vector.tensor_add(ot[:], F[t][:], bo_sb[:])
                        nc.sync.dma_start(out_t[:, t, :], ot[:])

    if repeat == 1 and mm_dt == MM_DT:
        _cached_nc = nc
    return nc


# ---------------------------------------------------------------------------
# Entry point
# ---------------------------------------------------------------------------
def _np_dt(mm_dt):
    if mm_dt == F16:
        return np.float16
    return np.float32


def make_in_maps(x, attn_mask, Wq, bq, Wk, bk, Wv, bv, Wo, bo, mm_dt=None):
    if mm_dt is None:
        mm_dt = MM_DT
    ndt = _np_dt(mm_dt)
    bqc = np.ascontiguousarray(np.asarray(bq, np.float32).reshape(NT, P).T)
    bkc = np.ascontiguousarray(np.asarray(bk, np.float32).reshape(NT, P).T)
    bo_eff = (
        np.asarray(bv, np.float64) @ np.asarray(Wo, np.float64)
        + np.asarray(bo, np.float64)
    ).astype(np.float32)
    wqc = np.asarray(Wq, np.float32).astype(ndt)
    wkc = np.asarray(Wk, np.float32).astype(ndt)
    wvc = np.asarray(Wv, np.float32).astype(ndt)
    woc = np.asarray(Wo, np.float32).astype(ndt)
    in_maps = []
    for n in range(N):
        in_maps.append(
            {
                "x_t": np.ascontiguousarray(np.asarray(x[n], np.float32).T).astype(ndt),
                "mask_t": np.ascontiguousarray(np.asarray(attn_mask[n]).T).astype(np.uint8),
                "wq": wqc, "wk": wkc, "wv": wvc, "wo": woc,
                "bqc": bqc, "bkc": bkc, "bo_eff": bo_eff,
            }
        )
    return in_maps


def kernel(x, attn_mask, Wq, bq, Wk, bk, Wv, bv, Wo, bo, **_):
    nc = _build()
    in_maps = make_in_maps(x, attn_mask, Wq, bq, Wk, bk, Wv, bv, Wo, bo)
    res = run_bass_kernel_spmd(nc, in_maps, list(range(N)))
    outs = np.stack([np.asarray(res.results[n]["out"]) for n in range(N)], axis=0)
    return outs.astype(np.float32)



# revision 2
# speedup vs baseline: 2.7590x; 2.7590x over previous
"""Trainium2 Bass kernel for batched multi-head attention, v2.

Same math as the baseline (one batch element per core, transposed
activations, fp16 matmuls) restructured so the ScalarE exp stream (~128us,
the largest non-tensor engine load) overlaps the projection matmuls instead
of serializing behind them:

  spine: per pair p -- logits (row-packed 64-contraction matmul pairs)
  feed exp (ACT) + mask-mult (DVE); attn@V for pair p-1 and the K/Q
  projection groups for pair p+1 fill the tensor stream between logits
  units, so ACT runs continuously from ~30us into the rep.

  - V projection runs first (prologue) so attn@V chunks are ready to be
    fillers from cycle 1 on.
  - attn@V is c-chunk-major into 1-bank PSUM accumulators: PSUM =
    2x logits units (4 banks) + attnV chunk (2) + projection acc (2).
  - softmax sums ride the ones-columns; h0 blocks are [v|1], h1 blocks
    [1|v] so both heads' chunk drains (including the sums row) land in
    the 128-partition range of two staging tiles.
  - normalization: sums bounce through DRAM for partition-broadcast,
    then one reciprocal + two multiplies per pair (no [1,N] DVE ops).
  - mask convert + bounces on gpsimd; V drain on DVE; ACT does exp only.
  - phase C computes outT = Wo^T @ oT (output transposed, host
    un-transposes) so wo streams through the same per-group weight-slice
    pool as wq/wk and the bias is a per-partition scalar.
"""

import numpy as np
from contextlib import ExitStack

import concourse.bass as bass
import concourse.mybir as mybir
import concourse.tile as tile
from concourse.vector_clock import ScopedClock
from concourse.bass_utils import run_bass_kernel_spmd

F32 = mybir.dt.float32
F16 = mybir.dt.float16
U8 = mybir.dt.uint8
Exp = mybir.ActivationFunctionType.Exp
MULT = mybir.AluOpType.mult
ADD = mybir.AluOpType.add

N, S, E, H, DK = 8, 1024, 1024, 16, 64
P = 128
NT = E // P
NPAIR = H // 2
MM_DT = F16

# ---------------------------------------------------------------------------
# Workaround: this walrus build supports at most ONE semaphore wait per
# instruction (same as baseline kernel).
# ---------------------------------------------------------------------------
_MAXW = 1
_orig_lower = tile.TileContext._lower_ordered_insts
_tilefix_installed = False


def _split_waits(ordered):
    for _bb, insts in ordered.items():
        out = []
        for inst in insts:
            si = inst.sync_info
            if si is not None and len(si.on_wait) > _MAXW:
                waits = list(si.on_wait)
                keep, extra = waits[:_MAXW], waits[_MAXW:]
                for i in range(0, len(extra), _MAXW):
                    out.append(
                        mybir.InstNoOp(
                            name=f"{inst.name}-ws{i}",
                            engine=inst.engine,
                            bass_nofuse=True,
                            sync_info=mybir.SyncInfo(
                                on_wait=extra[i : i + _MAXW], on_update=[]
                            ),
                        )
                    )
                inst.sync_info = mybir.SyncInfo(
                    on_wait=keep, on_update=list(si.on_update)
                )
            out.append(inst)
        insts[:] = out


def _patched_lower(self, ordered):
    _split_waits(ordered)
    return _orig_lower(self, ordered)


def _patched_drain_and_barrier(self, tick_clock, wait_clock):
    nc = self.nc
    drain_inst = nc.sync.drain()
    wait_clock.add_sem_waits(
        drain_inst.ins, ScopedClock({None: tick_clock.global_clock})
    )
    si = drain_inst.ins.sync_info
    waits = list(si.on_wait) if si is not None else []
    if len(waits) > _MAXW:
        drain_inst.ins.sync_info = mybir.SyncInfo(on_wait=[], on_update=[])
        for i in range(0, len(waits), _MAXW):
            nop = nc.sync.nop(nofuse=True)
            nop.ins.sync_info = mybir.SyncInfo(
                on_wait=waits[i : i + _MAXW], on_update=[]
            )
    nc.all_engine_barrier()
    popped = nc._tile_sem_poison_stack.pop()
    assert popped is self._sem_poison
    nc.clear_and_free_semaphores(list(self.sems.allocated().values()))
    nc.all_engine_barrier()


def _install_tilefix():
    global _tilefix_installed
    if not _tilefix_installed:
        tile.TileContext._lower_ordered_insts = _patched_lower
        tile.TileContext._drain_and_barrier = _patched_drain_and_barrier
        _tilefix_installed = True


# ---------------------------------------------------------------------------
# Kernel build
# ---------------------------------------------------------------------------
_cached_nc = None


def _build(repeat=1):
    global _cached_nc
    if _cached_nc is not None and repeat == 1:
        return _cached_nc
    _install_tilefix()

    nc = bass.Bass("TRN2", num_devices=N)

    x_t = nc.declare_dram_parameter("x_t", [E, S], MM_DT, isOutput=False)
    mask_t = nc.declare_dram_parameter("mask_t", [S, S], U8, isOutput=False)
    wq = nc.declare_dram_parameter("wq", [E, E], MM_DT, isOutput=False)
    wk = nc.declare_dram_parameter("wk", [E, E], MM_DT, isOutput=False)
    wv = nc.declare_dram_parameter("wv", [E, E], MM_DT, isOutput=False)
    wo = nc.declare_dram_parameter("wo", [E, E], MM_DT, isOutput=False)
    bqc = nc.declare_dram_parameter("bqc", [P, NT], F32, isOutput=False)
    bkc = nc.declare_dram_parameter("bkc", [P, NT], F32, isOutput=False)
    boc = nc.declare_dram_parameter("boc", [P, NT], F32, isOutput=False)
    out = nc.declare_dram_parameter("out", [E, S], F32, isOutput=True)

    def tiled(ap):
        return ap.rearrange("(t p) f -> p t f", p=P)

    x_tt = tiled(x_t.ap())
    mask_tt = tiled(mask_t.ap())
    w_t = {
        "q": tiled(wq.ap()),
        "k": tiled(wk.ap()),
        "v": tiled(wv.ap()),
        "o": tiled(wo.ap()),
    }
    out_t = tiled(out.ap())

    ZB = DK + 1  # per-head V block: [v(64) | 1]

    with tile.TileContext(nc) as tc, ExitStack() as ctx:
        # persistent pools, longest lifetime first (stack allocator)
        p_pers = ctx.enter_context(tc.tile_pool(name="pers", bufs=1))
        QT = p_pers.tile([P, NT, S], MM_DT)
        KT = p_pers.tile([P, NT, S], MM_DT)
        Vg = p_pers.tile([P, NT, H * (DK + 1)], F16)
        nm = p_pers.tile([P, NT, S], F16)
        oT = p_pers.tile([P, NPAIR, S], MM_DT)
        bq_sb = p_pers.tile([P, NT], F32)
        bk_sb = p_pers.tile([P, NT], F32)
        bo_sb = p_pers.tile([P, NT], F32)

        p_em = ctx.enter_context(tc.tile_pool(name="em", bufs=2))
        p_wv = ctx.enter_context(tc.tile_pool(name="wv", bufs=1))
        p_ws = ctx.enter_context(tc.tile_pool(name="ws", bufs=4))
        p_x = ctx.enter_context(tc.tile_pool(name="xt", bufs=1))
        p_stg = ctx.enter_context(tc.tile_pool(name="stg", bufs=2))
        p_rb = ctx.enter_context(tc.tile_pool(name="rb", bufs=1))
        p_mu = ctx.enter_context(tc.tile_pool(name="mu", bufs=1))
        p_out = ctx.enter_context(tc.tile_pool(name="outc", bufs=1))
        p_dram = ctx.enter_context(tc.tile_pool(name="drm", bufs=4, space="DRAM"))
        ps_L = ctx.enter_context(tc.tile_pool(name="psL", bufs=2, space="PSUM"))
        ps_O = ctx.enter_context(tc.tile_pool(name="psO", bufs=2, space="PSUM"))
        ps_A = ctx.enter_context(tc.tile_pool(name="psA", bufs=2, space="PSUM"))

        for rep in range(repeat):
            nc.sync.dma_start(bq_sb[:], bqc[:])
            nc.sync.dma_start(bk_sb[:], bkc[:])
            nc.sync.dma_start(bo_sb[:], boc[:])
            nc.gpsimd.memset(Vg[:, :, DK::ZB], 1.0)

            xT = p_x.tile([P, NT, S], MM_DT, tag="x", name=f"xT_{rep}")
            Wv_sb = p_wv.tile([P, NT, S], MM_DT, tag="wv", name=f"wv_{rep}")
            for j in range(NT):
                nc.sync.dma_start(xT[:, j, :], x_tt[:, j, :])
                nc.sync.dma_start(Wv_sb[:, j, :], w_t["v"][:, j, :])
            for j in range(NT):
                mu = p_mu.tile([P, S], U8, tag="mu", name=f"mu_{rep}_{j}")
                nc.sync.dma_start(mu[:], mask_tt[:, j, :])
                nc.gpsimd.tensor_scalar(
                    nm[:, j, :], mu[:], -1.0, 1.0, op0=MULT, op1=ADD
                )

            def w_slice(proj, t):
                Wg = p_ws.tile(
                    [P, NT, P], MM_DT, tag="ws", name=f"ws_{rep}_{proj}_{t}"
                )
                nc.sync.dma_start(Wg[:], w_t[proj][:, :, t * P : (t + 1) * P])
                return Wg

            # one (proj, t): full embed contraction into two 1-bank accs
            def a_group(proj, t, W):
                accs = [
                    ps_A.tile(
                        [P, 512], F32, tag="acc", name=f"acc_{rep}_{proj}_{t}_{c}"
                    )
                    for c in range(2)
                ]
                for j in range(NT):
                    for c in range(2):
                        if proj == "v":
                            lhsT = xT[:, j, t * P : (t + 1) * P]
                            rhs = W[:, j, c * 512 : (c + 1) * 512]
                        else:
                            lhsT = W[:, j, :]
                            rhs = xT[:, j, c * 512 : (c + 1) * 512]
                        nc.tensor.matmul(
                            accs[c][:], lhsT, rhs,
                            start=(j == 0), stop=(j == NT - 1),
                        )
                for c in range(2):
                    sl = slice(c * 512, (c + 1) * 512)
                    if proj == "q":
                        nc.vector.tensor_scalar_add(
                            QT[:, t, sl], accs[c][:], bq_sb[:, t : t + 1]
                        )
                    elif proj == "k":
                        nc.vector.tensor_scalar_add(
                            KT[:, t, sl], accs[c][:], bk_sb[:, t : t + 1]
                        )
                    else:
                        # seq-major V rows -> head-major [v|1] blocks
                        dst = Vg[:, t, :].rearrange("p (hd z) -> p hd z", z=ZB)
                        src = accs[c][:].rearrange("p (hd z) -> p hd z", z=DK)
                        hd = slice(c * 8, (c + 1) * 8)
                        nc.vector.tensor_copy(dst[:, hd, 0:DK], src[:])

            def logits_pair(p, j, Ls):
                # c-major, h-interleaved: the two 64-row tiles (T0/T8)
                # stream concurrently when adjacent in issue order
                for c in range(2):
                    for h in range(2):
                        r0, r1 = h * DK, (h + 1) * DK
                        nc.tensor.matmul(
                            Ls[h][:, c * 512 : (c + 1) * 512],
                            KT[r0:r1, p, j * P : (j + 1) * P],
                            QT[r0:r1, p, c * 512 : (c + 1) * 512],
                            start=True, stop=True,
                        )

            def attnv_chunk(p, h, c, Em, StgA, StgB):
                head = 2 * p + h
                Oc = ps_O.tile(
                    [DK + 1, 512], F32, tag="O", name=f"O_{rep}_{p}_{h}_{c}"
                )
                for j in range(NT):
                    nc.tensor.matmul(
                        Oc[:],
                        Vg[:, j, head * (DK + 1) : (head + 1) * (DK + 1)],
                        Em[:, j, h, c * 512 : (c + 1) * 512],
                        start=(j == 0), stop=(j == NT - 1),
                    )
                # h0 -> StgA rows 0..64 (sums at 64); h1 V -> StgB rows
                # 64..127 with its sums row parked at StgB row 0, so every
                # two-input op downstream has equal base partitions.
                if h == 0:
                    nc.vector.tensor_copy(StgA[0:65, c, :], Oc[:])
                else:
                    nc.vector.tensor_copy(StgB[DK:P, c, :], Oc[0:DK, :])
                    nc.vector.tensor_copy(StgB[0:1, c, :], Oc[DK : DK + 1, :])

            def normalize(p, StgA, StgB):
                Rd = p_dram.tile([1, 2, 2, 512], F16, tag="rd", name=f"rd_{rep}_{p}")
                nc.gpsimd.dma_start(Rd[:, 0], StgA[64:65, :, :])
                nc.gpsimd.dma_start(Rd[:, 1], StgB[0:1, :, :])
                Sb = p_rb.tile([P, S], F16, tag="sbb", name=f"sbb_{rep}_{p}")
                nc.gpsimd.dma_start(
                    Sb[0:DK, :],
                    Rd[:, 0].rearrange("o c q -> o (c q)").broadcast_to((DK, S)),
                )
                nc.gpsimd.dma_start(
                    Sb[DK:P, :],
                    Rd[:, 1].rearrange("o c q -> o (c q)").broadcast_to((DK, S)),
                )
                Rb = p_rb.tile([P, S], F32, tag="rbb", name=f"rbb_{rep}_{p}")
                nc.vector.reciprocal(Rb[:], Sb[:])
                nc.vector.tensor_tensor(
                    oT[0:DK, p, :],
                    StgA[0:DK, :, :].rearrange("p c q -> p (c q)"),
                    Rb[0:DK, :], MULT,
                )
                nc.vector.tensor_tensor(
                    oT[DK:P, p, :],
                    StgB[DK:P, :, :].rearrange("p c q -> p (c q)"),
                    Rb[DK:P, :], MULT,
                )

            # =========== schedule ===========
            # prologue: K0/Q0 only, so exp starts ~7us in; V groups run
            # as cycle-0 fillers (attn@V first needs them in cycle 1).
            kq = {}
            kq[("k", 0)] = w_slice("k", 0)
            kq[("q", 0)] = w_slice("q", 0)
            a_group("k", 0, kq[("k", 0)])
            a_group("q", 0, kq[("q", 0)])

            prev = None  # (p, Em, StgA, StgB)
            for p in range(NPAIR):
                Em = p_em.tile(
                    [P, NT, 2, S], F16, tag="em", name=f"em_{rep}_{p}"
                )
                StgA = p_stg.tile([P, 2, 512], F16, tag="sa", name=f"sa_{rep}_{p}")
                StgB = p_stg.tile([P, 2, 512], F16, tag="sb2", name=f"sb2_{rep}_{p}")
                # K/Q projections for the NEXT pair run as this cycle's
                # fillers (pair p's were done in the previous cycle).
                if p + 1 < NPAIR:
                    kq[("k", p + 1)] = w_slice("k", p + 1)
                    kq[("q", p + 1)] = w_slice("q", p + 1)
                for j in range(NT):
                    Ls = [
                        ps_L.tile(
                            [P, S], F32, tag="L", name=f"L_{rep}_{p}_{j}_{h}"
                        )
                        for h in range(2)
                    ]
                    logits_pair(p, j, Ls)
                    for h in range(2):
                        nc.scalar.activation(
                            Em[:, j, h, :], Ls[h][:], Exp, scale=0.125
                        )
                    nc.vector.tensor_tensor(
                        Em[:, j, :, :],
                        Em[:, j, :, :],
                        nm[:, j : j + 1, :].broadcast_to((P, 2, S)),
                        MULT,
                    )
                    # fillers
                    if p == 0:
                        a_group("v", j, Wv_sb)
                    if j in (1, 3, 5, 7) and prev is not None:
                        ph, pc = divmod((j - 1) // 2, 2)
                        attnv_chunk(prev[0], ph, pc, prev[1], prev[2], prev[3])
                    elif j == 2 and p + 1 < NPAIR:
                        a_group("k", p + 1, kq[("k", p + 1)])
                    elif j == 4 and p + 1 < NPAIR:
                        a_group("q", p + 1, kq[("q", p + 1)])
                if prev is not None:
                    normalize(prev[0], prev[2], prev[3])
                prev = (p, Em, StgA, StgB)

            for ph in range(2):
                for pc in range(2):
                    attnv_chunk(prev[0], ph, pc, prev[1], prev[2], prev[3])
            normalize(prev[0], prev[2], prev[3])

            # ---- phase C: outT = sum_j wo_j^T-slices @ oT_j ----
            for t in range(NT):
                Wog = w_slice("o", t)
                F = ps_L.tile([P, S], F32, tag="L", name=f"F_{rep}_{t}")
                for j in range(NPAIR):
                    for c in range(2):
                        nc.tensor.matmul(
                            F[:, c * 512 : (c + 1) * 512],
                            Wog[:, j, :],
                            oT[:, j, c * 512 : (c + 1) * 512],
                            start=(j == 0), stop=(j == NPAIR - 1),
                        )
                ot = p_out.tile([P, S], F32, tag="ot", name=f"ot_{rep}_{t}")
                nc.vector.tensor_scalar_add(ot[:], F[:], bo_sb[:, t : t + 1])
                nc.sync.dma_start(out_t[:, t, :], ot[:])

    if repeat == 1:
        _cached_nc = nc
    return nc


# ---------------------------------------------------------------------------
# Entry point
# ---------------------------------------------------------------------------
def make_in_maps(x, attn_mask, Wq, bq, Wk, bk, Wv, bv, Wo, bo):
    ndt = np.float16
    bqc = np.ascontiguousarray(np.asarray(bq, np.float32).reshape(NT, P).T)
    bkc = np.ascontiguousarray(np.asarray(bk, np.float32).reshape(NT, P).T)
    bo_eff = (
        np.asarray(bv, np.float64) @ np.asarray(Wo, np.float64)
        + np.asarray(bo, np.float64)
    ).astype(np.float32)
    boc = np.ascontiguousarray(bo_eff.reshape(NT, P).T)
    wqc = np.asarray(Wq, np.float32).astype(ndt)
    wkc = np.asarray(Wk, np.float32).astype(ndt)
    wvc = np.asarray(Wv, np.float32).astype(ndt)
    woc = np.asarray(Wo, np.float32).astype(ndt)
    in_maps = []
    for n in range(N):
        in_maps.append(
            {
                "x_t": np.ascontiguousarray(np.asarray(x[n], np.float32).T).astype(ndt),
                "mask_t": np.ascontiguousarray(np.asarray(attn_mask[n]).T).astype(np.uint8),
                "wq": wqc, "wk": wkc, "wv": wvc, "wo": woc,
                "bqc": bqc, "bkc": bkc, "boc": boc,
            }
        )
    return in_maps


def kernel(x, attn_mask, Wq, bq, Wk, bk, Wv, bv, Wo, bo, **_):
    nc = _build()
    in_maps = make_in_maps(x, attn_mask, Wq, bq, Wk, bk, Wv, bv, Wo, bo)
    res = run_bass_kernel_spmd(nc, in_maps, list(range(N)))
    outs = np.stack(
        [np.asarray(res.results[n]["out"]).T for n in range(N)], axis=0
    )
    return np.ascontiguousarray(outs).astype(np.float32)
